# revision 29
# baseline (speedup 1.0000x reference)
"""KAN-style spline layer (nn_BaseLayer_83425444757708) on 8 TRN2 NeuronCores.

Math: the reference evaluates, for every edge e = o*128 + i, the 11 cubic
B-spline basis functions of x[b, i] over a knot vector grid[e] (all edges
share one knot vector), contracts with c_basis, multiplies by c_spl, and adds
a SiLU residual path.

Because the knot vector is shared, each cubic B-spline basis function can be
written via divided differences of truncated powers.  With
D[j, t] = (t[j+4]-t[j]) / prod_{s != t} (t[t]-t[s]) restricted to the 5-knot
support window:

    basis_j(x) = sum_t D[j,t] * relu(x - t_t)^3

(right-sided powers work with the same D because a 4th-order divided
difference annihilates the cubic polynomial part).  The D-combination and the
c_spl/c_basis contraction fold into one weight matrix on the host, so the
device work collapses to

    out[b,o] = sum_{i,t} W2[o,i,t] * relu(x[b,i] - t_t)^3 + silu(x) @ c_res.T

i.e. 16 K-tiles (15 truncated-power tiles + 1 silu tile) of a single
(512 x 2048) @ (2048 x 128) matmul, with the activations built on-chip as
    sq = (x - t)^2        (scalar engine, Square activation)
    r  = max(x - t, 0)    (vector engine, tensor_scalar add+max)
    s3 = sq * r           (gpsimd/vector tensor_tensor)

Sharding: batch split in 2, contraction split in 4 -> 8 cores, host sums the
4 K-partials per batch half.  The SPMD program is identical on every core:
knot offsets and weights arrive as data (bias columns appended to the x pack,
zero-weight pad tile, silu/residual weights only on the last K-shard).
"""

import os

import numpy as np

B_TOT, N_IN, N_OUT = 512, 128, 128
NKNOTS, NBASIS, KDEG = 15, 11, 3
B_SHARD, K_SHARD = 2, 4
N_CORES = B_SHARD * K_SHARD
CB = B_TOT // B_SHARD                      # batch rows per core
N_SP = -(-NKNOTS // K_SHARD)               # spline K-tiles per core (padded)
N_STEPS = N_SP + 1                         # + silu tile

IMPL = os.environ.get("KERNEL_IMPL", "raw")          # "raw" | "tile"
CLEAR_SEMS = os.environ.get("KERNEL_CLEAR_SEMS", "0") == "1"
MM_DTYPE = os.environ.get("KERNEL_MM_DTYPE", "f32")  # "f32" | "f32r"

_prog_cache = {}
LAST_RESULT = None  # BassKernelResults of the most recent device run


def _ensure_ntff_hook():
    """This image's ``antenv`` lacks ``axon_hooks``, so NTFF profiling under
    axon silently degrades.  Register the ctypes-based hook ourselves so
    BASS_TRACE=1 produces a profile; harmless no-op if anything is missing."""
    import sys
    import types

    if "antenv.axon_hooks" in sys.modules:
        return
    try:
        import antenv
        from trn_agent_boot.trn_boot import _ntff_profile_via_ctypes

        hook = _ntff_profile_via_ctypes("/opt/axon/libaxon_pjrt.so")
        mod = types.ModuleType("antenv.axon_hooks")
        mod._hook = hook
        mod.set_axon_ntff_profile_hook = lambda h: setattr(mod, "_hook", h)
        mod.get_axon_ntff_profile_hook = lambda: mod._hook
        sys.modules["antenv.axon_hooks"] = mod
        antenv.axon_hooks = mod
    except Exception:
        pass


def _build_raw(cb, n_sp, mm_dtype):
    """Raw (non-Tile, non-Block) program: one basic block, explicit per-engine
    streams and semaphores.

    TileContext costs ~10us of fixed overhead (entry EVSEM sync, tail drain +
    EVSEM butterfly) and even ``nc.Block`` emits entry/exit all-engine
    barriers (~7us).  Here every instruction is emitted straight into the main
    block; every cross-engine dependency is one explicit semaphore wait; the
    sync engine clears all semaphores at the very end (safe: the final
    out-DMA wait transitively proves every other engine has retired), which
    keeps the NEFF re-executable.

    Engine split per K-shard (n_sp spline tiles + silu):
      scalar : silu (first, so one act-table load covers silu+square), then
               sq_l = (x - t_l)^2 via Square activation
      vector : r_l = max(x - t_l, 0) via tensor_scalar, cubes for the last
               two tiles, psum -> sbuf copy
      gpsimd : cubes for the first two tiles
      tensor : 5 accumulating matmuls (weights stationary, batch moving)
      sync   : weight DMA + output DMA (x pack DMA goes on the scalar
               engine's separate HWDGE ring for overlap)
    """
    from contextlib import ExitStack

    import concourse.bacc as bacc
    import concourse.mybir as mybir

    f32 = mybir.dt.float32
    mmdt = mybir.dt.float32r if mm_dtype == "f32r" else mybir.dt.float32
    AFT = mybir.ActivationFunctionType
    ALU = mybir.AluOpType
    n_steps = n_sp + 1
    n_warm = int(os.environ.get("KERNEL_N_WARM", "0"))  # HAM warmup matmuls

    nc = bacc.Bacc()

    # Strip the Bass.__init__ preamble: const-AP memsets (we never use const
    # APs — every bias/scale is data or an immediate) and the boot all-engine
    # barrier (drain + event-semaphore per engine, ~3us of serialized boot
    # skew).  Nothing in this straight-line kernel needs engines aligned at
    # entry; all cross-engine deps carry explicit semaphores.
    for bb in nc.m.functions[0].blocks:
        for ins in [
            i
            for i in bb.instructions
            if type(i).__name__ in ("InstMemset", "InstDrain", "InstEventSemaphore")
        ]:
            bb.instructions.remove(ins)

    # Force one activation-table load: restrict the candidate act-func sets to
    # those covering every function we use, so the insert_act_table_loads pass
    # picks a single covering set (index positions preserved).
    if not hasattr(bacc, "_orig_get_activation_tables"):
        bacc._orig_get_activation_tables = bacc.get_activation_tables

        def _covering_tables(arch):
            tabs = bacc._orig_get_activation_tables(arch)
            need = {AFT.Silu, AFT.Square}
            return {n: (s if need <= s else set()) for n, s in tabs.items()}

        bacc.get_activation_tables = _covering_tables

    xp = nc.declare_dram_parameter("xp", [128, cb + n_sp + 1], f32, isOutput=False)
    wp = nc.declare_dram_parameter("wp", [128, n_steps * 128], mmdt, isOutput=False)
    outT = nc.declare_dram_parameter("outT", [128, cb], f32, isOutput=True)

    ctx = ExitStack()
    with ctx:
        XT = ctx.enter_context(nc.sbuf_tensor("XT", [128, cb + n_sp + 1], f32))
        W = ctx.enter_context(nc.sbuf_tensor("W", [128, n_steps * 128], mmdt))
        SQ = [
            ctx.enter_context(nc.sbuf_tensor(f"SQ{l}", [128, cb], f32))
            for l in range(n_sp)
        ]
        R = [
            ctx.enter_context(nc.sbuf_tensor(f"R{l}", [128, cb], f32))
            for l in range(n_sp)
        ]
        S3 = [
            ctx.enter_context(nc.sbuf_tensor(f"S3{l}", [128, cb], mmdt))
            for l in range(n_sp)
        ]
        SIL = ctx.enter_context(nc.sbuf_tensor("SIL", [128, cb], mmdt))
        OT = ctx.enter_context(nc.sbuf_tensor("OT", [128, cb], f32))
        PS = ctx.enter_context(nc.psum_tensor("PS", [128, cb], f32))

        d_x = ctx.enter_context(nc.semaphore("d_x"))
        d_w = ctx.enter_context(nc.semaphore("d_w"))
        d_o = ctx.enter_context(nc.semaphore("d_o"))
        s_act = ctx.enter_context(nc.semaphore("s_act"))
        s_rel = ctx.enter_context(nc.semaphore("s_rel"))
        s_gp = ctx.enter_context(nc.semaphore("s_gp"))
        s_dve = ctx.enter_context(nc.semaphore("s_dve"))
        s_pe = ctx.enter_context(nc.semaphore("s_pe"))
        s_cp = ctx.enter_context(nc.semaphore("s_cp"))
        all_sems = [d_x, d_w, d_o, s_act, s_rel, s_dve, s_pe, s_cp]

        xin = XT[:, 0:cb]

        def bias_ap(l):            # -t_l for l < n_sp; 0.0 at l == n_sp (silu)
            return XT[:, cb + l : cb + l + 1]

        # ---- scalar engine: x DMA on the ACT HWDGE ring, then activations.
        # Squares first (they gate the cube muls and the matmul chain); silu
        # last (only the final matmul needs it).  s_act counts sq_0..sq_3
        # then silu.
        nc.scalar.dma_start(out=XT[:], in_=xp[:]).then_inc(d_x, 16)
        nc.scalar.wait_ge(d_x, 16)
        for l in range(n_sp):
            nc.scalar.activation(
                SQ[l][:], xin, AFT.Square, bias=bias_ap(l), scale=1.0
            ).then_inc(s_act, 1)
        nc.scalar.activation(
            SIL[:], xin, AFT.Silu, bias=bias_ap(n_sp), scale=1.0
        ).then_inc(s_act, 1)

        # ---- sync engine: weight DMA, then the two output half DMAs + sem
        # cleanup (safe: d_o>=32 transitively proves every engine retired)
        nc.sync.dma_start(out=W[:], in_=wp[:]).then_inc(d_w, 16)
        nc.sync.wait_ge(s_cp, 1)
        nc.sync.dma_start(out=outT[:], in_=OT[:]).then_inc(d_o, 16)
        nc.sync.wait_ge(d_o, 16)
        if CLEAR_SEMS:
            for sem in all_sems:
                nc.sync.sem_clear(sem)

        # ---- vector engine: relu / cube-mul interleaved (earliest s3 for PE),
        # then the psum->sbuf copy.  GpSimd is intentionally unused: its
        # 2-input ops are ~5x slower and port-share against the DVE.
        nc.vector.wait_ge(d_x, 16)
        for l in range(n_sp):
            nc.vector.tensor_scalar(
                R[l][:], xin, bias_ap(l), 0.0, ALU.add, ALU.max
            ).then_inc(s_rel, 1)
            nc.vector.wait_ge(s_act, l + 1)               # sq_l ready
            nc.vector.wait_ge(s_rel, l + 1)               # own r_l retired (deep pipe)
            nc.vector.tensor_mul(S3[l][:], SQ[l][:], R[l][:]).then_inc(s_dve, 1)
        nc.vector.wait_ge(s_pe, 1)
        nc.vector.tensor_copy(OT[:], PS[:]).then_inc(s_cp, 1)

        # ---- tensor engine: HAM warmup on junk data while waiting for the
        # weight DMA (a cold PE runs fp32 matmuls at half clock), then the
        # accumulating matmul chain; the final (silu) step is split into two
        # batch halves so the copy/out-DMA tail overlaps it.
        nc.tensor.wait_ge(d_w, 16)
        for l in range(n_sp):
            nc.tensor.wait_ge(s_dve, l + 1)
            nc.tensor.matmul(
                PS[:],
                lhsT=W[:, l * 128 : (l + 1) * 128],
                rhs=S3[l][:],
                start=(l == 0),
                stop=False,
            )
        nc.tensor.wait_ge(s_act, n_sp + 1)
        nc.tensor.matmul(
            PS[:],
            lhsT=W[:, n_sp * 128 : (n_sp + 1) * 128],
            rhs=SIL[:],
            start=False,
            stop=True,
        ).then_inc(s_pe, 1)

    nc.finalize()
    return nc


def _build_tile(cb, n_sp):
    """TileContext implementation (first working version; slower fixed costs)."""
    import concourse.bacc as bacc
    import concourse.mybir as mybir
    from concourse import tile

    f32 = mybir.dt.float32
    AFT = mybir.ActivationFunctionType
    n_steps = n_sp + 1
    n_m = (cb + 127) // 128

    nc = bacc.Bacc()
    xT = nc.declare_dram_parameter("xT", [N_IN, cb], f32, isOutput=False)
    wp = nc.declare_dram_parameter("wp", [128, n_steps * 128], f32, isOutput=False)
    ct = nc.declare_dram_parameter("ct", [128, n_sp], f32, isOutput=False)
    out = nc.declare_dram_parameter("out", [cb, N_OUT], f32, isOutput=True)

    with tile.TileContext(nc) as tc:
        with (
            tc.tile_pool(name="sbuf", bufs=1) as pool,
            tc.tile_pool(name="psum", bufs=1, space="PSUM") as pp,
        ):
            xt = pool.tile([N_IN, cb], f32, tag="xt")
            nc.sync.dma_start(out=xt[:], in_=xT[:])
            w = pool.tile([128, n_steps * 128], f32, tag="w")
            nc.sync.dma_start(out=w[:], in_=wp[:])
            c = pool.tile([128, n_sp], f32, tag="c")
            nc.sync.dma_start(out=c[:], in_=ct[:])

            psums = []
            for mb in range(n_m):
                mm = min(128, cb - mb * 128)
                psums.append(pp.tile([mm, N_OUT], f32, tag=f"ps{mb}", name=f"ps{mb}"))

            prime = pp.tile([1, 1], f32, tag="prime", name="prime")
            nc.tensor.matmul(prime[:], lhsT=w[:, 0:1], rhs=w[:, 0:1], start=True, stop=True)

            for l in range(n_sp):
                r = pool.tile([N_IN, cb], f32, tag=f"r{l}")
                nc.scalar.activation(
                    r[:], xt[:], AFT.Relu, bias=c[:, l : l + 1], scale=-1.0
                )
                r2 = pool.tile([N_IN, cb], f32, tag=f"r2_{l}")
                nc.scalar.activation(r2[:], r[:], AFT.Square)
                s3 = pool.tile([N_IN, cb], f32, tag=f"s3_{l}")
                nc.vector.tensor_mul(s3[:], r2[:], r[:])
                for mb in range(n_m):
                    mm = min(128, cb - mb * 128)
                    nc.tensor.matmul(
                        psums[mb][:],
                        lhsT=s3[:, mb * 128 : mb * 128 + mm],
                        rhs=w[:, l * 128 : (l + 1) * 128],
                        start=(l == 0),
                        stop=False,
                    )

            sl = pool.tile([N_IN, cb], f32, tag="sl")
            nc.scalar.activation(sl[:], xt[:], AFT.Silu)
            for mb in range(n_m):
                mm = min(128, cb - mb * 128)
                nc.tensor.matmul(
                    psums[mb][:],
                    lhsT=sl[:, mb * 128 : mb * 128 + mm],
                    rhs=w[:, n_sp * 128 : (n_sp + 1) * 128],
                    start=False,
                    stop=True,
                )

            for mb in range(n_m):
                mm = min(128, cb - mb * 128)
                o = pool.tile([mm, N_OUT], f32, tag=f"o{mb}")
                nc.vector.tensor_copy(o[:], psums[mb][:])
                nc.sync.dma_start(out=out[mb * 128 : mb * 128 + mm, :], in_=o[:])
    nc.finalize()
    return nc


def _dd_weights(knots):
    """D[j, t] such that basis_j(x) = sum_t D[j,t] * relu(x - knots[t])^3."""
    D = np.zeros((NBASIS, NKNOTS))
    for j in range(NBASIS):
        pts = knots[j : j + 5]
        for r in range(5):
            denom = 1.0
            for s in range(5):
                if s != r:
                    denom *= pts[r] - pts[s]
            D[j, j + r] = (knots[j + 4] - knots[j]) / denom
    return D


def _numpy_fallback(x, grid, c_basis, c_res, c_spl):
    """Direct Cox-de Boor replication for inputs outside the shared-knot fast
    path (never hit for this problem's generator; correctness net only)."""
    x64 = x.astype(np.float64)
    out = np.zeros((x.shape[0], N_OUT), np.float64)
    silu = x64 / (1.0 + np.exp(-x64))
    out += silu @ c_res.T.astype(np.float64)
    g = grid.astype(np.float64)
    for o in range(N_OUT):
        acc = np.zeros((x.shape[0], N_IN), np.float64)
        for i in range(N_IN):
            e = o * N_IN + i
            xe = x64[:, i][None, :]
            ge = g[e][:, None]
            b = ((xe >= ge[:-1]) & (xe < ge[1:])).astype(np.float64)
            for Kd in range(1, KDEG + 1):
                left = (xe - ge[: -(Kd + 1)]) / (ge[Kd:-1] - ge[: -(Kd + 1)])
                right = (ge[Kd + 1 :] - xe) / (ge[Kd + 1 :] - ge[1:-Kd])
                b = left * b[:-1] + right * b[1:]
            acc[:, i] = c_basis[e].astype(np.float64) @ b
        out[:, o] += (acc * c_spl[o][None, :].astype(np.float64)).sum(axis=1)
    return out.astype(np.float32)


def kernel(x, grid, c_basis, c_res, c_spl):
    global LAST_RESULT
    x = np.asarray(x, np.float32)
    grid = np.asarray(grid, np.float32)
    c_basis = np.asarray(c_basis, np.float32)
    c_res = np.asarray(c_res, np.float32)
    c_spl = np.asarray(c_spl, np.float32)

    if not (grid == grid[0]).all() or not (np.diff(grid[0]) > 0).all():
        return _numpy_fallback(x, grid, c_basis, c_res, c_spl)

    knots = grid[0].astype(np.float64)
    D = _dd_weights(knots)                                   # (11, 15)
    W = c_spl[:, :, None].astype(np.float64) * c_basis.reshape(
        N_OUT, N_IN, NBASIS
    ).astype(np.float64)                                     # (O, I, 11)
    W2 = np.einsum("oij,jt->tio", W, D)                      # (15, I, O)
    W2 = np.ascontiguousarray(W2, np.float32)

    # relu(x - t_t)^3 is identically zero on the data when t_t >= max(x), so
    # those truncated-power tiles contribute nothing and are dropped (for the
    # generator's x ~ U[0,1) that removes knots 1.0..1.375: 15 -> 11 tiles).
    x_max = float(x.max())
    active = [t for t in range(NKNOTS) if knots[t] < x_max]
    n_sp = max(1, -(-len(active) // K_SHARD))
    n_steps = n_sp + 1

    key = (IMPL, MM_DTYPE, CB, n_sp)
    if key not in _prog_cache:
        if IMPL == "raw":
            _prog_cache[key] = _build_raw(CB, n_sp, MM_DTYPE)
        else:
            _prog_cache[key] = _build_tile(CB, n_sp)
    nc = _prog_cache[key]

    # K-shard kb owns active tiles [kb*n_sp, ...); the last shard is padded
    # with zero tiles and carries the silu/residual weights.
    in_maps = []
    for core in range(N_CORES):
        bb, kb = divmod(core, K_SHARD)
        xT_c = np.ascontiguousarray(x[bb * CB : (bb + 1) * CB, :].T)
        wp_c = np.zeros((128, n_steps * 128), np.float32)
        biases = np.zeros(n_sp, np.float32)
        for l in range(n_sp):
            ti = kb * n_sp + l
            if ti < len(active):
                t = active[ti]
                wp_c[:, l * 128 : (l + 1) * 128] = W2[t]
                biases[l] = -knots[t]
        if kb == K_SHARD - 1:
            wp_c[:, n_sp * 128 :] = c_res.T
        if IMPL == "raw":
            bias_cols = np.zeros((128, n_sp + 1), np.float32)
            bias_cols[:, :n_sp] = biases
            xp_c = np.ascontiguousarray(
                np.concatenate([xT_c, bias_cols], axis=1).astype(np.float32)
            )
            in_maps.append({"xp": xp_c, "wp": wp_c})
        else:
            ct_c = np.broadcast_to(biases * -1.0, (128, n_sp)).astype(np.float32)
            # tile impl uses relu(-x + t) with bias=+t
            in_maps.append({"xT": xT_c, "wp": wp_c, "ct": np.ascontiguousarray(ct_c)})

    _ensure_ntff_hook()
    from concourse.bass_utils import run_bass_kernel_spmd

    LAST_RESULT = run_bass_kernel_spmd(nc, in_maps, list(range(N_CORES)))

    acc = np.zeros((B_TOT, N_OUT), np.float64)
    for core in range(N_CORES):
        bb = core // K_SHARD
        if IMPL == "raw":
            acc[bb * CB : (bb + 1) * CB] += LAST_RESULT.results[core]["outT"].T
        else:
            acc[bb * CB : (bb + 1) * CB] += LAST_RESULT.results[core]["out"]
    return acc.astype(np.float32)


# revision 30
# speedup vs baseline: 1.1012x; 1.1012x over previous
"""KAN-style spline layer (nn_BaseLayer_83425444757708) on 8 TRN2 NeuronCores.

Math: the reference evaluates, for every edge e = o*128 + i, the 11 cubic
B-spline basis functions of x[b, i] over a knot vector grid[e] (all edges
share one knot vector), contracts with c_basis, multiplies by c_spl, and adds
a SiLU residual path.

Because the knot vector is shared, each cubic B-spline basis function can be
written via divided differences of truncated powers.  With
D[j, t] = (t[j+4]-t[j]) / prod_{s != t} (t[t]-t[s]) restricted to the 5-knot
support window:

    basis_j(x) = sum_t D[j,t] * relu(x - t_t)^3

(right-sided powers work with the same D because a 4th-order divided
difference annihilates the cubic polynomial part).  The D-combination and the
c_spl/c_basis contraction fold into one weight matrix on the host, so the
device work collapses to

    out[b,o] = sum_{i,t} W2[o,i,t] * relu(x[b,i] - t_t)^3 + silu(x) @ c_res.T

i.e. 16 K-tiles (15 truncated-power tiles + 1 silu tile) of a single
(512 x 2048) @ (2048 x 128) matmul, with the activations built on-chip as
    sq = (x - t)^2        (scalar engine, Square activation)
    r  = max(x - t, 0)    (vector engine, tensor_scalar add+max)
    s3 = sq * r           (gpsimd/vector tensor_tensor)

Sharding: batch split in 2, contraction split in 4 -> 8 cores, host sums the
4 K-partials per batch half.  The SPMD program is identical on every core:
knot offsets and weights arrive as data (bias columns appended to the x pack,
zero-weight pad tile, silu/residual weights only on the last K-shard).
"""

import os

import numpy as np

B_TOT, N_IN, N_OUT = 512, 128, 128
NKNOTS, NBASIS, KDEG = 15, 11, 3
B_SHARD, K_SHARD = 2, 4
N_CORES = B_SHARD * K_SHARD
CB = B_TOT // B_SHARD                      # batch rows per core
N_SP = -(-NKNOTS // K_SHARD)               # spline K-tiles per core (padded)
N_STEPS = N_SP + 1                         # + silu tile

IMPL = os.environ.get("KERNEL_IMPL", "raw")          # "raw" | "tile"
CLEAR_SEMS = os.environ.get("KERNEL_CLEAR_SEMS", "0") == "1"
WAIT_DMA_OUT = os.environ.get("KERNEL_WAIT_DMA_OUT", "0") == "1"
MM_DTYPE = os.environ.get("KERNEL_MM_DTYPE", "f32")  # "f32" | "f32r"

_prog_cache = {}
LAST_RESULT = None  # BassKernelResults of the most recent device run


def _ensure_ntff_hook():
    """This image's ``antenv`` lacks ``axon_hooks``, so NTFF profiling under
    axon silently degrades.  Register the ctypes-based hook ourselves so
    BASS_TRACE=1 produces a profile; harmless no-op if anything is missing."""
    import sys
    import types

    if "antenv.axon_hooks" in sys.modules:
        return
    try:
        import antenv
        from trn_agent_boot.trn_boot import _ntff_profile_via_ctypes

        hook = _ntff_profile_via_ctypes("/opt/axon/libaxon_pjrt.so")
        mod = types.ModuleType("antenv.axon_hooks")
        mod._hook = hook
        mod.set_axon_ntff_profile_hook = lambda h: setattr(mod, "_hook", h)
        mod.get_axon_ntff_profile_hook = lambda: mod._hook
        sys.modules["antenv.axon_hooks"] = mod
        antenv.axon_hooks = mod
    except Exception:
        pass


def _build_raw(cb, n_sp, mm_dtype):
    """Raw (non-Tile, non-Block) program: one basic block, explicit per-engine
    streams and semaphores.

    TileContext costs ~10us of fixed overhead (entry EVSEM sync, tail drain +
    EVSEM butterfly) and even ``nc.Block`` emits entry/exit all-engine
    barriers (~7us).  Here every instruction is emitted straight into the main
    block; every cross-engine dependency is one explicit semaphore wait; the
    sync engine clears all semaphores at the very end (safe: the final
    out-DMA wait transitively proves every other engine has retired), which
    keeps the NEFF re-executable.

    Engine split per K-shard (n_sp spline tiles + silu):
      scalar : silu (first, so one act-table load covers silu+square), then
               sq_l = (x - t_l)^2 via Square activation
      vector : r_l = max(x - t_l, 0) via tensor_scalar, cubes for the last
               two tiles, psum -> sbuf copy
      gpsimd : cubes for the first two tiles
      tensor : 5 accumulating matmuls (weights stationary, batch moving)
      sync   : weight DMA + output DMA (x pack DMA goes on the scalar
               engine's separate HWDGE ring for overlap)
    """
    from contextlib import ExitStack

    import concourse.bacc as bacc
    import concourse.mybir as mybir

    f32 = mybir.dt.float32
    mmdt = mybir.dt.float32r if mm_dtype == "f32r" else mybir.dt.float32
    AFT = mybir.ActivationFunctionType
    ALU = mybir.AluOpType
    n_steps = n_sp + 1
    n_warm = int(os.environ.get("KERNEL_N_WARM", "0"))  # HAM warmup matmuls

    nc = bacc.Bacc()

    # Strip the Bass.__init__ preamble: const-AP memsets (we never use const
    # APs — every bias/scale is data or an immediate) and the boot all-engine
    # barrier (drain + event-semaphore per engine, ~3us of serialized boot
    # skew).  Nothing in this straight-line kernel needs engines aligned at
    # entry; all cross-engine deps carry explicit semaphores.
    for bb in nc.m.functions[0].blocks:
        for ins in [
            i
            for i in bb.instructions
            if type(i).__name__ in ("InstMemset", "InstDrain", "InstEventSemaphore")
        ]:
            bb.instructions.remove(ins)

    # Force one activation-table load: restrict the candidate act-func sets to
    # those covering every function we use, so the insert_act_table_loads pass
    # picks a single covering set (index positions preserved).
    if not hasattr(bacc, "_orig_get_activation_tables"):
        bacc._orig_get_activation_tables = bacc.get_activation_tables

        def _covering_tables(arch):
            tabs = bacc._orig_get_activation_tables(arch)
            need = {AFT.Silu, AFT.Square}
            return {n: (s if need <= s else set()) for n, s in tabs.items()}

        bacc.get_activation_tables = _covering_tables

    xp = nc.declare_dram_parameter("xp", [128, cb + n_sp + 1], f32, isOutput=False)
    wp = nc.declare_dram_parameter("wp", [128, n_steps * 128], mmdt, isOutput=False)
    outT = nc.declare_dram_parameter("outT", [128, cb], f32, isOutput=True)

    ctx = ExitStack()
    with ctx:
        XT = ctx.enter_context(nc.sbuf_tensor("XT", [128, cb + n_sp + 1], f32))
        W = ctx.enter_context(nc.sbuf_tensor("W", [128, n_steps * 128], mmdt))
        SQ = [
            ctx.enter_context(nc.sbuf_tensor(f"SQ{l}", [128, cb], f32))
            for l in range(n_sp)
        ]
        R = [
            ctx.enter_context(nc.sbuf_tensor(f"R{l}", [128, cb], f32))
            for l in range(n_sp)
        ]
        S3 = [
            ctx.enter_context(nc.sbuf_tensor(f"S3{l}", [128, cb], mmdt))
            for l in range(n_sp)
        ]
        SIL = ctx.enter_context(nc.sbuf_tensor("SIL", [128, cb], mmdt))
        OT = ctx.enter_context(nc.sbuf_tensor("OT", [128, cb], f32))
        PS = ctx.enter_context(nc.psum_tensor("PS", [128, cb], f32))

        d_x = ctx.enter_context(nc.semaphore("d_x"))
        d_w = ctx.enter_context(nc.semaphore("d_w"))
        d_o = ctx.enter_context(nc.semaphore("d_o"))
        s_act = ctx.enter_context(nc.semaphore("s_act"))
        s_rel = ctx.enter_context(nc.semaphore("s_rel"))
        s_gp = ctx.enter_context(nc.semaphore("s_gp"))
        s_dve = ctx.enter_context(nc.semaphore("s_dve"))
        s_pe = ctx.enter_context(nc.semaphore("s_pe"))
        s_cp = ctx.enter_context(nc.semaphore("s_cp"))
        all_sems = [d_x, d_w, d_o, s_act, s_rel, s_dve, s_pe, s_cp]

        xin = XT[:, 0:cb]

        def bias_ap(l):            # -t_l for l < n_sp; 0.0 at l == n_sp (silu)
            return XT[:, cb + l : cb + l + 1]

        # ---- scalar engine: x DMA on the ACT HWDGE ring, then activations.
        # Squares first (they gate the cube muls and the matmul chain); silu
        # last (only the final matmul needs it).  s_act counts sq_0..sq_3
        # then silu.
        nc.scalar.dma_start(out=XT[:], in_=xp[:]).then_inc(d_x, 16)
        nc.scalar.wait_ge(d_x, 16)
        for l in range(n_sp):
            nc.scalar.activation(
                SQ[l][:], xin, AFT.Square, bias=bias_ap(l), scale=1.0
            ).then_inc(s_act, 1)
        nc.scalar.activation(
            SIL[:], xin, AFT.Silu, bias=bias_ap(n_sp), scale=1.0
        ).then_inc(s_act, 1)

        # ---- sync engine: weight DMA, then the two output half DMAs + sem
        # cleanup (safe: d_o>=32 transitively proves every engine retired)
        nc.sync.dma_start(out=W[:], in_=wp[:]).then_inc(d_w, 16)
        nc.sync.wait_ge(s_cp, 1)
        nc.sync.dma_start(out=outT[:], in_=OT[:]).then_inc(d_o, 16)
        if WAIT_DMA_OUT:
            nc.sync.wait_ge(d_o, 16)
        if CLEAR_SEMS:
            for sem in all_sems:
                nc.sync.sem_clear(sem)

        # ---- vector engine: relu / cube-mul interleaved (earliest s3 for PE),
        # then the psum->sbuf copy.  GpSimd is intentionally unused: its
        # 2-input ops are ~5x slower and port-share against the DVE.
        nc.vector.wait_ge(d_x, 16)
        for l in range(n_sp):
            nc.vector.tensor_scalar(
                R[l][:], xin, bias_ap(l), 0.0, ALU.add, ALU.max
            ).then_inc(s_rel, 1)
            nc.vector.wait_ge(s_act, l + 1)               # sq_l ready
            nc.vector.wait_ge(s_rel, l + 1)               # own r_l retired (deep pipe)
            nc.vector.tensor_mul(S3[l][:], SQ[l][:], R[l][:]).then_inc(s_dve, 1)
        nc.vector.wait_ge(s_pe, 1)
        nc.vector.tensor_copy(OT[:], PS[:]).then_inc(s_cp, 1)

        # ---- tensor engine: HAM warmup on junk data while waiting for the
        # weight DMA (a cold PE runs fp32 matmuls at half clock), then the
        # accumulating matmul chain; the final (silu) step is split into two
        # batch halves so the copy/out-DMA tail overlaps it.
        nc.tensor.wait_ge(d_w, 16)
        for l in range(n_sp):
            nc.tensor.wait_ge(s_dve, l + 1)
            nc.tensor.matmul(
                PS[:],
                lhsT=W[:, l * 128 : (l + 1) * 128],
                rhs=S3[l][:],
                start=(l == 0),
                stop=False,
            )
        nc.tensor.wait_ge(s_act, n_sp + 1)
        nc.tensor.matmul(
            PS[:],
            lhsT=W[:, n_sp * 128 : (n_sp + 1) * 128],
            rhs=SIL[:],
            start=False,
            stop=True,
        ).then_inc(s_pe, 1)

    nc.finalize()
    return nc


def _build_tile(cb, n_sp):
    """TileContext implementation (first working version; slower fixed costs)."""
    import concourse.bacc as bacc
    import concourse.mybir as mybir
    from concourse import tile

    f32 = mybir.dt.float32
    AFT = mybir.ActivationFunctionType
    n_steps = n_sp + 1
    n_m = (cb + 127) // 128

    nc = bacc.Bacc()
    xT = nc.declare_dram_parameter("xT", [N_IN, cb], f32, isOutput=False)
    wp = nc.declare_dram_parameter("wp", [128, n_steps * 128], f32, isOutput=False)
    ct = nc.declare_dram_parameter("ct", [128, n_sp], f32, isOutput=False)
    out = nc.declare_dram_parameter("out", [cb, N_OUT], f32, isOutput=True)

    with tile.TileContext(nc) as tc:
        with (
            tc.tile_pool(name="sbuf", bufs=1) as pool,
            tc.tile_pool(name="psum", bufs=1, space="PSUM") as pp,
        ):
            xt = pool.tile([N_IN, cb], f32, tag="xt")
            nc.sync.dma_start(out=xt[:], in_=xT[:])
            w = pool.tile([128, n_steps * 128], f32, tag="w")
            nc.sync.dma_start(out=w[:], in_=wp[:])
            c = pool.tile([128, n_sp], f32, tag="c")
            nc.sync.dma_start(out=c[:], in_=ct[:])

            psums = []
            for mb in range(n_m):
                mm = min(128, cb - mb * 128)
                psums.append(pp.tile([mm, N_OUT], f32, tag=f"ps{mb}", name=f"ps{mb}"))

            prime = pp.tile([1, 1], f32, tag="prime", name="prime")
            nc.tensor.matmul(prime[:], lhsT=w[:, 0:1], rhs=w[:, 0:1], start=True, stop=True)

            for l in range(n_sp):
                r = pool.tile([N_IN, cb], f32, tag=f"r{l}")
                nc.scalar.activation(
                    r[:], xt[:], AFT.Relu, bias=c[:, l : l + 1], scale=-1.0
                )
                r2 = pool.tile([N_IN, cb], f32, tag=f"r2_{l}")
                nc.scalar.activation(r2[:], r[:], AFT.Square)
                s3 = pool.tile([N_IN, cb], f32, tag=f"s3_{l}")
                nc.vector.tensor_mul(s3[:], r2[:], r[:])
                for mb in range(n_m):
                    mm = min(128, cb - mb * 128)
                    nc.tensor.matmul(
                        psums[mb][:],
                        lhsT=s3[:, mb * 128 : mb * 128 + mm],
                        rhs=w[:, l * 128 : (l + 1) * 128],
                        start=(l == 0),
                        stop=False,
                    )

            sl = pool.tile([N_IN, cb], f32, tag="sl")
            nc.scalar.activation(sl[:], xt[:], AFT.Silu)
            for mb in range(n_m):
                mm = min(128, cb - mb * 128)
                nc.tensor.matmul(
                    psums[mb][:],
                    lhsT=sl[:, mb * 128 : mb * 128 + mm],
                    rhs=w[:, n_sp * 128 : (n_sp + 1) * 128],
                    start=False,
                    stop=True,
                )

            for mb in range(n_m):
                mm = min(128, cb - mb * 128)
                o = pool.tile([mm, N_OUT], f32, tag=f"o{mb}")
                nc.vector.tensor_copy(o[:], psums[mb][:])
                nc.sync.dma_start(out=out[mb * 128 : mb * 128 + mm, :], in_=o[:])
    nc.finalize()
    return nc


def _dd_weights(knots):
    """D[j, t] such that basis_j(x) = sum_t D[j,t] * relu(x - knots[t])^3."""
    D = np.zeros((NBASIS, NKNOTS))
    for j in range(NBASIS):
        pts = knots[j : j + 5]
        for r in range(5):
            denom = 1.0
            for s in range(5):
                if s != r:
                    denom *= pts[r] - pts[s]
            D[j, j + r] = (knots[j + 4] - knots[j]) / denom
    return D


def _numpy_fallback(x, grid, c_basis, c_res, c_spl):
    """Direct Cox-de Boor replication for inputs outside the shared-knot fast
    path (never hit for this problem's generator; correctness net only)."""
    x64 = x.astype(np.float64)
    out = np.zeros((x.shape[0], N_OUT), np.float64)
    silu = x64 / (1.0 + np.exp(-x64))
    out += silu @ c_res.T.astype(np.float64)
    g = grid.astype(np.float64)
    for o in range(N_OUT):
        acc = np.zeros((x.shape[0], N_IN), np.float64)
        for i in range(N_IN):
            e = o * N_IN + i
            xe = x64[:, i][None, :]
            ge = g[e][:, None]
            b = ((xe >= ge[:-1]) & (xe < ge[1:])).astype(np.float64)
            for Kd in range(1, KDEG + 1):
                left = (xe - ge[: -(Kd + 1)]) / (ge[Kd:-1] - ge[: -(Kd + 1)])
                right = (ge[Kd + 1 :] - xe) / (ge[Kd + 1 :] - ge[1:-Kd])
                b = left * b[:-1] + right * b[1:]
            acc[:, i] = c_basis[e].astype(np.float64) @ b
        out[:, o] += (acc * c_spl[o][None, :].astype(np.float64)).sum(axis=1)
    return out.astype(np.float32)


def kernel(x, grid, c_basis, c_res, c_spl):
    global LAST_RESULT
    x = np.asarray(x, np.float32)
    grid = np.asarray(grid, np.float32)
    c_basis = np.asarray(c_basis, np.float32)
    c_res = np.asarray(c_res, np.float32)
    c_spl = np.asarray(c_spl, np.float32)

    if not (grid == grid[0]).all() or not (np.diff(grid[0]) > 0).all():
        return _numpy_fallback(x, grid, c_basis, c_res, c_spl)

    knots = grid[0].astype(np.float64)
    D = _dd_weights(knots)                                   # (11, 15)
    W = c_spl[:, :, None].astype(np.float64) * c_basis.reshape(
        N_OUT, N_IN, NBASIS
    ).astype(np.float64)                                     # (O, I, 11)
    W2 = np.einsum("oij,jt->tio", W, D)                      # (15, I, O)
    W2 = np.ascontiguousarray(W2, np.float32)

    # relu(x - t_t)^3 is identically zero on the data when t_t >= max(x), so
    # those truncated-power tiles contribute nothing and are dropped (for the
    # generator's x ~ U[0,1) that removes knots 1.0..1.375: 15 -> 11 tiles).
    x_max = float(x.max())
    active = [t for t in range(NKNOTS) if knots[t] < x_max]
    n_sp = max(1, -(-len(active) // K_SHARD))
    n_steps = n_sp + 1

    key = (IMPL, MM_DTYPE, CB, n_sp)
    if key not in _prog_cache:
        if IMPL == "raw":
            _prog_cache[key] = _build_raw(CB, n_sp, MM_DTYPE)
        else:
            _prog_cache[key] = _build_tile(CB, n_sp)
    nc = _prog_cache[key]

    # K-shard kb owns active tiles [kb*n_sp, ...); the last shard is padded
    # with zero tiles and carries the silu/residual weights.
    in_maps = []
    for core in range(N_CORES):
        bb, kb = divmod(core, K_SHARD)
        xT_c = np.ascontiguousarray(x[bb * CB : (bb + 1) * CB, :].T)
        wp_c = np.zeros((128, n_steps * 128), np.float32)
        biases = np.zeros(n_sp, np.float32)
        for l in range(n_sp):
            ti = kb * n_sp + l
            if ti < len(active):
                t = active[ti]
                wp_c[:, l * 128 : (l + 1) * 128] = W2[t]
                biases[l] = -knots[t]
        if kb == K_SHARD - 1:
            wp_c[:, n_sp * 128 :] = c_res.T
        if IMPL == "raw":
            bias_cols = np.zeros((128, n_sp + 1), np.float32)
            bias_cols[:, :n_sp] = biases
            xp_c = np.ascontiguousarray(
                np.concatenate([xT_c, bias_cols], axis=1).astype(np.float32)
            )
            in_maps.append({"xp": xp_c, "wp": wp_c})
        else:
            ct_c = np.broadcast_to(biases * -1.0, (128, n_sp)).astype(np.float32)
            # tile impl uses relu(-x + t) with bias=+t
            in_maps.append({"xT": xT_c, "wp": wp_c, "ct": np.ascontiguousarray(ct_c)})

    _ensure_ntff_hook()
    from concourse.bass_utils import run_bass_kernel_spmd

    LAST_RESULT = run_bass_kernel_spmd(nc, in_maps, list(range(N_CORES)))

    acc = np.zeros((B_TOT, N_OUT), np.float64)
    for core in range(N_CORES):
        bb = core // K_SHARD
        if IMPL == "raw":
            acc[bb * CB : (bb + 1) * CB] += LAST_RESULT.results[core]["outT"].T
        else:
            acc[bb * CB : (bb + 1) * CB] += LAST_RESULT.results[core]["out"]
    return acc.astype(np.float32)
